# revision 1
# baseline (speedup 1.0000x reference)
"""GAT multi-head attention (nn_GATMHAEfficient) on 8 Trainium2 NeuronCores.

Strategy (data-parallel over batch B=32 -> 4 graphs per core):
  Host folds W/Wal/War into one weight matrix Wcat (128 x 152):
    columns [h*17 .. h*17+15] = W[h] (16 cols), column h*17+16 = 0 (later
    memset to 1.0 on-chip -> the "ones" column that makes the aggregation
    matmul also produce the softmax denominator), columns 136..144 = W@Wal
    per head (gives a_i directly from h), columns 144..152 = W@War (a_j).
  Per graph b:  X = h_b @ Wcat  on PE  ->  g, a_i, a_j in one pass.
  Per (b, head): scores are built in (j, i) layout so the aggregation
  matmul can stream them as the moving operand (f32r -> 1 PE cycle/row):
    t   = mask^T * (-1e30) + broadcast(a_i) [+ a_j]
    t   = leaky_relu(t [+ a_j])    alpha=0.2   (ACT Prelu or DVE STT)
    P   = Exp(t)                               (ACT, full tile)
    U^T = sum_c  [g|1]_c^T @ P_c               (PE, PSUM accumulate)
    out = relu(U[:, 0:16] * (1/U[:, 16]))      (PE transpose + DVE)
  The elementwise score work is the bottleneck, so it is load-balanced
  across ACT / DVE / POOL with a per-(b,h) "flavor":
    F1: mask+bcast merge on DVE,  leaky on ACT
    F2: mask+bcast merge on POOL, leaky on ACT
    F3: bcast+aj on DVE(2x), mask merge on POOL, leaky on DVE
"""

import json

import numpy as np

import concourse.bass as bass
import concourse.mybir as mybir
import concourse.tile as tile
from concourse.vector_clock import ScopedClock, VectorClock

F32 = mybir.dt.float32
F32R = mybir.dt.float32r
U8 = mybir.dt.uint8
BF16 = mybir.dt.bfloat16
AF = mybir.ActivationFunctionType
ALU = mybir.AluOpType

B, N, NI, H, D = 32, 1024, 128, 8, 16
NCORES = 8
B_SH = B // NCORES          # graphs per core
C = N // 128                # j-chunks of 128
NEG_SLOPE = 0.2
WCOLS = H * (D + 1) + 2 * H  # 152
MASK_PEN = -1.0e30

# flavor pattern over the 32 (b,h) pairs; tuned from TimelineSim profiles
def _mk_flavors():
    # flavor = (who adds a_j, who does leaky, who applies the mask)
    # F4 : fused into ACT prelu | ACT | POOL
    # F6 : fused into ACT prelu | ACT | DVE
    # F5 : DVE ts               | DVE | POOL
    # F3 : DVE ts               | DVE | DVE
    base = ["F4", "F5", "F6", "F5", "F4", "F3", "F5", "F4",
            "F5", "F3", "F4", "F6", "F5", "F4", "F5", "F3"]
    return [base[i % len(base)] for i in range(32)]

FLAVOR_PATTERN = _mk_flavors()

# ---------------------------------------------------------------------------
# Workarounds for this container's walrus build: it accepts at most ONE
# sync-wait per instruction, but Tile's sem-assignment (and its final drain)
# attach several. Split the excess onto dedicated single-wait EventSemaphore
# carrier instructions in the serialized BIR.


def _legalize_sync_waits(d, max_waits=1):
    for fn in d["functions"]:
        for bb in fn["blocks"]:
            new_insts = []
            for inst in bb["instructions"]:
                si = inst.get("sync_info") or {}
                w = si.get("on_wait") or []
                if len(w) > max_waits:
                    for k, we in enumerate(w[:-max_waits]):
                        new_insts.append(
                            {
                                "debug": inst.get("debug", 0),
                                "engine": inst["engine"],
                                "ins": [],
                                "outs": [],
                                "name": f"{inst['name']}_xw{k}",
                                "opcode": "EventSemaphore",
                                "sync_info": {"on_update": [], "on_wait": [we]},
                            }
                        )
                    si["on_wait"] = w[-max_waits:]
                new_insts.append(inst)
            bb["instructions"] = new_insts


def _wrap_to_json(nc):
    raw = nc.to_json_bytes

    def patched():
        d = json.loads(raw())
        _legalize_sync_waits(d)
        return json.dumps(d).encode()

    nc.to_json_bytes = patched


def _split_drain_and_barrier(self, tick_clock, wait_clock):
    # One drain per logical processor so each carries a single sem wait.
    gc = tick_clock.global_clock
    n = len(gc)
    for proc in range(n):
        t = gc[proc]
        if t > 0:
            dr = self.nc.sync.drain()
            pc = VectorClock([t if i == proc else 0 for i in range(n)])
            wait_clock.add_sem_waits(dr.ins, ScopedClock({None: pc}))
    self.nc.all_engine_barrier()
    popped = self.nc._tile_sem_poison_stack.pop()
    assert popped is self._sem_poison
    self.nc.clear_and_free_semaphores(list(self.sems.allocated().values()))
    self.nc.all_engine_barrier()


tile.TileContext._drain_and_barrier = _split_drain_and_barrier

# ---------------------------------------------------------------------------


def _bcast_rep_ap(t, reps):
    """View a [128, F] tile as [128, reps, F] with a 0-stride middle dim."""
    return bass.AP(tensor=t.tensor, offset=t.offset, ap=[t.ap[0], [0, reps], t.ap[-1]])


def build_nc():
    nc = bass.Bass()
    hT = nc.dram_tensor("hT", [B_SH, NI, N], F32, kind="ExternalInput")
    notmT = nc.dram_tensor("notmT", [B_SH, N, N], BF16, kind="ExternalInput")
    wcat = nc.dram_tensor("wcat", [NI, WCOLS], F32, kind="ExternalInput")
    id17 = nc.dram_tensor("id17", [D + 1, D + 1], F32, kind="ExternalInput")
    out = nc.dram_tensor("out", [B_SH, N, H * D], F32, kind="ExternalOutput")
    ai_scr = nc.dram_tensor("ai_scr", [B_SH, H, N], F32)  # internal scratch

    from contextlib import ExitStack

    with ExitStack() as ctx:
        tc = ctx.enter_context(tile.TileContext(nc))
        const_p = ctx.enter_context(tc.tile_pool(name="const", bufs=1))
        hb_p = ctx.enter_context(tc.tile_pool(name="hb", bufs=2))
        xs_p = ctx.enter_context(tc.tile_pool(name="xs", bufs=2))
        ai_p = ctx.enter_context(tc.tile_pool(name="ai", bufs=2))
        bc_p = ctx.enter_context(tc.tile_pool(name="bc", bufs=4))
        sc_p = ctx.enter_context(tc.tile_pool(name="sc", bufs=7))
        ut_p = ctx.enter_context(tc.tile_pool(name="ut", bufs=2))
        rc_p = ctx.enter_context(tc.tile_pool(name="rc", bufs=2))
        ob_p = ctx.enter_context(tc.tile_pool(name="ob", bufs=2))
        nm_p = ctx.enter_context(tc.tile_pool(name="nm", bufs=2))
        xps_p = ctx.enter_context(tc.tile_pool(name="xps", bufs=2, space="PSUM"))
        aps_p = xps_p
        vps_p = xps_p
        ups_p = ctx.enter_context(tc.tile_pool(name="ups", bufs=2, space="PSUM"))
        if True:
            wcat_s = const_p.tile([NI, WCOLS], F32)
            nc.sync.dma_start(out=wcat_s[:], in_=wcat[:])
            id17_s = const_p.tile([D + 1, D + 1], F32)
            nc.sync.dma_start(out=id17_s[:], in_=id17[:])
            onec = const_p.tile([128, 1], F32)
            nc.vector.memset(onec[:], 1.0)

            def _prep(b):
                hbT = hb_p.tile([NI, N], F32)
                nc.sync.dma_start(out=hbT[:], in_=hT[b])
                notm = nm_p.tile([128, C, N], BF16)
                nc.sync.dma_start(
                    out=notm[:], in_=notmT[b].rearrange("(c p) i -> p c i", p=128)
                )

                # X = h_b @ Wcat, one 128-row chunk at a time.
                # g columns land in an f32r tile (the aggregation matmul
                # requires f32r-rounded producers); a_j columns stay fp32.
                GEXT = H * (D + 1)  # 136
                gext_r = xs_p.tile([128, C, GEXT], F32R, tag="gext")
                Xs_aj = xs_p.tile([128, C, H], F32, tag="xsaj")
                for c in range(C):
                    X_ps = xps_p.tile([128, WCOLS], F32, tag="xv")
                    nc.tensor.matmul(
                        X_ps[:],
                        lhsT=hbT[:, c * 128 : (c + 1) * 128],
                        rhs=wcat_s[:],
                        start=True,
                        stop=True,
                    )
                    nc.vector.tensor_copy(gext_r[:, c, :], X_ps[:, 0:GEXT])
                    nc.scalar.copy(out=Xs_aj[:, c, :], in_=X_ps[:, GEXT + H :])
                # ones column per head block (the denominator column of
                # gext); memset can't write f32r, so broadcast-copy from an
                # fp32 constant instead
                ones_view = bass.AP(
                    tensor=gext_r.tensor,
                    offset=gext_r.offset + D,  # first ones slot at col 16
                    ap=[gext_r.ap[0], [GEXT, C], [D + 1, H]],
                )
                ones_src = bass.AP(
                    tensor=onec.tensor,
                    offset=onec.offset,
                    ap=[onec.ap[0], [0, C], [0, H]],
                )
                nc.vector.tensor_copy(ones_view, ones_src)

                # a_i rows for every head: (W@Wal)^T @ h_b -> (8, N),
                # round-tripped through DRAM so each row can be broadcast
                # to all partitions by a replicating DMA.
                XT_ps = aps_p.tile([H, N], F32, tag="xv")
                for half in range(2):
                    sl = slice(half * 512, (half + 1) * 512)
                    nc.tensor.matmul(
                        XT_ps[:, sl],
                        lhsT=wcat_s[:, H * (D + 1) : H * (D + 1) + H],
                        rhs=hbT[:, sl],
                        start=True,
                        stop=True,
                    )
                ais8 = ai_p.tile([H, N], F32)
                nc.vector.tensor_copy(ais8[:], XT_ps[:])
                nc.sync.dma_start(out=ai_scr[b], in_=ais8[:])

                return notm, gext_r, Xs_aj

            preps = {0: _prep(0)}

            for b in range(B_SH):
                notm, gext_r, Xs_aj = preps.pop(b)
                out_b = ob_p.tile([128, C, H * D], F32)

                def postproc(h, UT_ps, flavor):
                        UT_s = ut_p.tile([D + 1, N], F32)
                        if h % 2 == 0:
                            nc.vector.tensor_copy(UT_s[:], UT_ps[:])
                        else:
                            nc.scalar.copy(out=UT_s[:], in_=UT_ps[:])

                        # transpose back to (i, 17) and normalize
                        V_ps = vps_p.tile([128, C, D + 1], F32, tag="xv")
                        for c in range(C):
                            nc.tensor.transpose(
                                V_ps[:, c, :],
                                UT_s[:, c * 128 : (c + 1) * 128],
                                id17_s[:],
                            )
                        rc_s = rc_p.tile([128, C], F32)
                        nc.vector.reciprocal(rc_s[:], V_ps[:, :, D])
                        # batched: u = V * (1/S) broadcast along d, then relu
                        rc_b = bass.AP(
                            tensor=rc_s.tensor,
                            offset=rc_s.offset,
                            ap=[rc_s.ap[0], rc_s.ap[-1], [0, D]],
                        )
                        u_s = rc_p.tile([128, C, D], F32, tag="u_s")
                        nc.vector.tensor_tensor(
                            out=u_s[:], in0=V_ps[:, :, 0:D], in1=rc_b, op=ALU.mult
                        )
                        nc.vector.tensor_scalar(
                            out=out_b[:, :, h * D : (h + 1) * D],
                            in0=u_s[:],
                            scalar1=0.0,
                            scalar2=None,
                            op0=ALU.max,
                        )


                pending = []

                for h in range(H):

                    # broadcast a_i over all 128 partitions (replicating DMA)
                    bc = bc_p.tile([128, N], F32)
                    bcast_src = bass.AP(
                        tensor=ai_scr,
                        offset=(b * H + h) * N,
                        ap=[[0, 128], [1, N]],
                    )
                    nc.sync.dma_start(out=bc[:], in_=bcast_src)

                    def aj_col(c):
                        return bass.AP(
                            tensor=Xs_aj.tensor,
                            offset=Xs_aj.offset + c * H + h,
                            ap=[Xs_aj.ap[0], [1, 1]],
                        )

                    UT_ps = ups_p.tile([D + 1, N], F32)
                    gh = gext_r[:, :, h * (D + 1) : (h + 1) * (D + 1)]
                    # stage-major emission in half-pair groups: each engine
                    # streams its stage back-to-back so chunk stages pipeline
                    GRP = 2
                    for g0 in range(0, C, GRP):
                        cs = list(range(g0, g0 + GRP))
                        flavor = FLAVOR_PATTERN[
                            ((b * H + h) * (C // GRP) + g0 // GRP) % len(FLAVOR_PATTERN)
                        ]
                        # one 2-chunk tile: per-chunk APs for the bias stages,
                        # full-tile (FD 2048) for leaky/exp/mask to halve the
                        # per-op init overhead on ACT/DVE/POOL
                        t2c = sc_p.tile([128, GRP, N], F32, tag="sc", name=f"t_{b}_{h}_{g0}")
                        if flavor in ("F4", "F6"):
                            # t = leaky_relu(bc + a_j) straight from bc on ACT
                            for i, c in enumerate(cs):
                                nc.scalar.activation(
                                    out=t2c[:, i, :],
                                    in_=bc[:],
                                    func=AF.Prelu,
                                    bias=aj_col(c),
                                    scale=1.0,
                                    alpha=NEG_SLOPE,
                                )
                        else:
                            # t = bc + a_j (DVE 2x), then leaky in place (DVE)
                            for i, c in enumerate(cs):
                                nc.vector.tensor_scalar(
                                    out=t2c[:, i, :],
                                    in0=bc[:],
                                    scalar1=aj_col(c),
                                    scalar2=None,
                                    op0=ALU.add,
                                )
                            nc.vector.scalar_tensor_tensor(
                                out=t2c[:],
                                in0=t2c[:],
                                scalar=NEG_SLOPE,
                                in1=t2c[:],
                                op0=ALU.mult,
                                op1=ALU.max,
                            )
                        # P = exp(...) in place  (unmasked), full 2-chunk op
                        nc.scalar.activation(out=t2c[:], in_=t2c[:], func=AF.Exp)
                        # P_m = P * (1 - mask), rounding to f32r, full 2-chunk
                        meng = nc.gpsimd if flavor in ("F4", "F5") else nc.vector
                        pm2 = sc_p.tile([128, GRP, N], F32R, tag="pm", name=f"pm_{b}_{h}_{g0}")
                        meng.tensor_tensor(
                            out=pm2[:],
                            in0=t2c[:],
                            in1=notm[:, g0 : g0 + GRP, :],
                            op=ALU.mult,
                        )
                        for i, c in enumerate(cs):
                            for half in range(2):
                                sl = slice(half * 512, (half + 1) * 512)
                                nc.tensor.matmul(
                                    UT_ps[:, sl],
                                    lhsT=gh[:, c, :],
                                    rhs=pm2[:, i, sl],
                                    start=(c == 0),
                                    stop=(c == C - 1),
                                )
                    # deferred postproc of the previous head overlaps
                    # this head's score stages
                    if pending:
                        postproc(*pending.pop())
                    pending.append((h, UT_ps, None))
                    # emit next graph's prep early so it overlaps this
                    # graph's remaining heads instead of stalling at the
                    # boundary
                    if h == 1 and b + 1 < B_SH:
                        preps[b + 1] = _prep(b + 1)

                if pending:
                    postproc(*pending.pop())
                nc.sync.dma_start(
                    out=out[b].rearrange("(c p) d -> p c d", p=128), in_=out_b[:]
                )

    _wrap_to_json(nc)
    return nc


_NC_CACHE = None


def kernel(h, W, Wal, War, mask):
    global _NC_CACHE
    from concourse.bass_utils import run_bass_kernel_spmd

    h = np.asarray(h, dtype=np.float32)
    W = np.asarray(W, dtype=np.float32)
    Wal = np.asarray(Wal, dtype=np.float32)
    War = np.asarray(War, dtype=np.float32)
    import ml_dtypes

    notm_b16 = (~np.asarray(mask, dtype=bool)).astype(ml_dtypes.bfloat16)

    # Fold weights: wcat = [per-head (W_h | 0)] + [W@Wal] + [W@War]
    wcat = np.zeros((NI, WCOLS), dtype=np.float32)
    for hh in range(H):
        wcat[:, hh * (D + 1) : hh * (D + 1) + D] = W[hh]
        wcat[:, H * (D + 1) + hh] = W[hh] @ Wal[hh, :, 0]
        wcat[:, H * (D + 1) + H + hh] = W[hh] @ War[hh, :, 0]

    hT = np.ascontiguousarray(h.transpose(0, 2, 1))            # (B, I, N)
    notmT = np.ascontiguousarray(notm_b16.transpose(0, 2, 1))  # (B, j, i)
    id17 = np.eye(D + 1, dtype=np.float32)

    if _NC_CACHE is None:
        _NC_CACHE = build_nc()
    nc = _NC_CACHE

    in_maps = []
    for core in range(NCORES):
        sl = slice(core * B_SH, (core + 1) * B_SH)
        in_maps.append(
            {
                "hT": np.ascontiguousarray(hT[sl]),
                "notmT": np.ascontiguousarray(notmT[sl]),
                "wcat": wcat,
                "id17": id17,
            }
        )

    res = run_bass_kernel_spmd(nc, in_maps, list(range(NCORES)))
    out = np.concatenate([res.results[i]["out"] for i in range(NCORES)], axis=0)
    return out.astype(np.float32)



# revision 24
# speedup vs baseline: 1.4237x; 1.4237x over previous
"""GAT multi-head attention (nn_GATMHAEfficient) on 8 Trainium2 NeuronCores.

Data-parallel over batch B=32 -> 4 graphs per core.

Algorithm (exact piecewise-rank-1 decomposition of the GAT scores):
  e_ij = leaky_relu(a_i + a_j), P = exp(e) * notm, out = (P @ g) / (P @ 1).
  Since exp is monotone and leaky_relu is piecewise linear,
     exp(leaky(s)) = E_i*E_j   if s >= 0      (E = exp(a))
                   = F_i*F_j   if s <  0      (F = exp(0.2 a))
  so with the 0/1 indicator Z_ij = 1[a_i + a_j >= 0] and Zm = Z * notm:
     P @ [g|1] = diag(E_i) Zm @ (E_j [g|1]_j)
               + diag(F_i) (notm - Zm) @ (F_j [g|1]_j)
  The notm @ (F [g|1]) term is HEAD-SHARED: one moving pass of the mask
  covers 4 heads (68 stationary columns), 2 passes for all 8.
  Per (b,h) the only N x N device work is:
    pass1: Z = (bc + a_j) is_ge 0    DVE tensor_scalar, bf16 -> 4x mode
    pass2: Zm = Z * notm             tensor_tensor bf16 (DVE 2x / Pool)
  and one bf16 moving matmul stream Zm @ [E g|E | -F g|-F] (34 cols).
  An ACT-lane variant computes Z via Sign(bc + a_j) on the Activation
  engine, merged with mask via  Zm = 1[sign >= (2 - notm)].
"""

import json

import numpy as np

import concourse.bass as bass
import concourse.mybir as mybir
import concourse.tile as tile
from concourse.vector_clock import ScopedClock, VectorClock

F32 = mybir.dt.float32
F32R = mybir.dt.float32r
BF16 = mybir.dt.bfloat16
FP8 = mybir.dt.float8e4
AF = mybir.ActivationFunctionType
ALU = mybir.AluOpType

B, N, NI, H, D = 32, 1024, 128, 8, 16
NCORES = 8
B_SH = B // NCORES          # graphs per core
C = N // 128                # j-chunks of 128
NEG_SLOPE = 0.2
GEXT = H * (D + 1)          # 136: per-head [g(16)|1] blocks
WCOLS = GEXT + 2 * H        # 152: + a_i cols + a_j cols
DP1 = D + 1                 # 17
KSC = 16384.0               # power-of-2 logit prescale (sigmoid sharpness)
DEBUG_DUMP = False

G = 2                       # chunks per pipeline group
NSLOT = B_SH * H * (C // G)  # 128 (pair, group) slots

# per-slot (pass1 engine, merge engine): D=DVE ts / A=ACT sign; D=DVE tt / P=Pool tt
# per-pair UT-copy engine; tuned against TimelineSim
def _mk_flavors():
    base = ["DD", "AD", "DP", "DD", "AP", "DD", "AD", "DP",
            "DD", "DP", "DD", "AD", "DD", "AP", "DD", "DP"]
    return [base[i % len(base)] for i in range(NSLOT)]

FLAVORS = _mk_flavors()
COPY_ENG = [("A", "D", "A", "A", "A", "A", "A", "D")[i % 8] for i in range(B_SH * H)]
# second slot: engine for SBUF-only postproc ops (Pool cannot touch PSUM)
PP_ENG = ["P", "D", "P", "P"]

# ---------------------------------------------------------------------------
# Workarounds for this container's walrus build: it accepts at most ONE
# sync-wait per instruction, but Tile's sem-assignment (and its final drain)
# attach several. Split the excess onto dedicated single-wait EventSemaphore
# carrier instructions in the serialized BIR.


def _legalize_sync_waits(d, max_waits=1):
    for fn in d["functions"]:
        for bb in fn["blocks"]:
            new_insts = []
            for inst in bb["instructions"]:
                si = inst.get("sync_info") or {}
                w = si.get("on_wait") or []
                if len(w) > max_waits:
                    for k, we in enumerate(w[:-max_waits]):
                        new_insts.append(
                            {
                                "debug": inst.get("debug", 0),
                                "engine": inst["engine"],
                                "ins": [],
                                "outs": [],
                                "name": f"{inst['name']}_xw{k}",
                                "opcode": "EventSemaphore",
                                "sync_info": {"on_update": [], "on_wait": [we]},
                            }
                        )
                    si["on_wait"] = w[-max_waits:]
                new_insts.append(inst)
            bb["instructions"] = new_insts


def _wrap_to_json(nc):
    raw = nc.to_json_bytes

    def patched():
        d = json.loads(raw())
        _legalize_sync_waits(d)
        return json.dumps(d).encode()

    nc.to_json_bytes = patched


def _split_drain_and_barrier(self, tick_clock, wait_clock):
    # One drain per logical processor so each carries a single sem wait.
    gc = tick_clock.global_clock
    n = len(gc)
    for proc in range(n):
        t = gc[proc]
        if t > 0:
            dr = self.nc.sync.drain()
            pc = VectorClock([t if i == proc else 0 for i in range(n)])
            wait_clock.add_sem_waits(dr.ins, ScopedClock({None: pc}))
    self.nc.all_engine_barrier()
    popped = self.nc._tile_sem_poison_stack.pop()
    assert popped is self._sem_poison
    self.nc.clear_and_free_semaphores(list(self.sems.allocated().values()))
    self.nc.all_engine_barrier()


tile.TileContext._drain_and_barrier = _split_drain_and_barrier

# ---------------------------------------------------------------------------


def _rep_ap(t, offset_elems, part_ap, dims):
    """Build a broadcast AP view on tile t: dims is a list of [stride, count]."""
    return bass.AP(tensor=t.tensor, offset=t.offset + offset_elems,
                   ap=[part_ap] + dims)


def build_nc():
    nc = bass.Bass()
    hT = nc.dram_tensor("hT", [B_SH, NI, N], F32, kind="ExternalInput")
    notmT = nc.dram_tensor("notmT", [B_SH, N, N], BF16, kind="ExternalInput")
    wcat = nc.dram_tensor("wcat", [NI, WCOLS], F32, kind="ExternalInput")
    id34 = nc.dram_tensor("id34", [2 * DP1, 2 * DP1], F32, kind="ExternalInput")
    id68 = nc.dram_tensor("id68", [4 * DP1, 4 * DP1], F32, kind="ExternalInput")
    out = nc.dram_tensor("out", [B_SH, N, H * D], F32, kind="ExternalOutput")
    dbg_vm1 = (
        nc.dram_tensor("dbg_vm1", [B_SH, 128, C, 2, 4 * DP1], F32,
                       kind="ExternalOutput")
        if DEBUG_DUMP else None
    )
    ai_scr = nc.dram_tensor("ai_scr", [B_SH, H, N], BF16)  # internal scratch

    from contextlib import ExitStack

    with ExitStack() as ctx:
        tc = ctx.enter_context(tile.TileContext(nc))
        const_p = ctx.enter_context(tc.tile_pool(name="const", bufs=1))
        hb_p = ctx.enter_context(tc.tile_pool(name="hb", bufs=2))
        nm_p = ctx.enter_context(tc.tile_pool(name="nm", bufs=2))
        gx_p = ctx.enter_context(tc.tile_pool(name="gx", bufs=2))
        ss_p = ctx.enter_context(tc.tile_pool(name="ss", bufs=2))
        ef_p = ctx.enter_context(tc.tile_pool(name="ef", bufs=2))
        m1_p = ctx.enter_context(tc.tile_pool(name="m1", bufs=2))
        bc_p = ctx.enter_context(tc.tile_pool(name="bc", bufs=9))
        sc_p = ctx.enter_context(tc.tile_pool(name="sc", bufs=8))
        ut_p = ctx.enter_context(tc.tile_pool(name="ut", bufs=3))
        pp_p = ctx.enter_context(tc.tile_pool(name="pp", bufs=3))
        ob_p = ctx.enter_context(tc.tile_pool(name="ob", bufs=2))
        # PSUM budget (8 banks): prep shares one 2-bank buffer (serialized),
        # UT accumulators 2 banks x2, V transposes 1 bank x2.
        xps_p = ctx.enter_context(tc.tile_pool(name="xps", bufs=1, space="PSUM"))
        ups_p = ctx.enter_context(tc.tile_pool(name="ups", bufs=2, space="PSUM"))
        vps_p = ctx.enter_context(tc.tile_pool(name="vps", bufs=2, space="PSUM"))

        wcat_s = const_p.tile([NI, WCOLS], F32)
        nc.sync.dma_start(out=wcat_s[:], in_=wcat[:])
        id34_s = const_p.tile([2 * DP1, 2 * DP1], F32)
        nc.sync.dma_start(out=id34_s[:], in_=id34[:])
        id68_s = const_p.tile([4 * DP1, 4 * DP1], F32)
        nc.sync.dma_start(out=id68_s[:], in_=id68[:])
        onec = const_p.tile([128, 1], F32)
        nc.vector.memset(onec[:], 1.0)

        def _prep(b):
            """Graph-level prep: inputs, X-proj, E/F, stationaries, M1."""
            hbT = hb_p.tile([NI, N], F32)
            nc.sync.dma_start(out=hbT[:], in_=hT[b])
            notm = nm_p.tile([128, C, N], BF16, tag="nm")
            nc.sync.dma_start(
                out=notm[:], in_=notmT[b].rearrange("(c p) i -> p c i", p=128)
            )

            # X = h_b @ Wcat -> g (f32r, with ones cols), a_i rows, a_j cols
            gext_r = gx_p.tile([128, C, GEXT], F32, tag="gext")
            Xs_aj = gx_p.tile([128, C, H], F32, tag="xsaj")
            for c0 in range(0, C, 2):
                X_ps = xps_p.tile([128, 2, WCOLS], F32, tag="prep")
                for i in range(2):
                    c = c0 + i
                    nc.tensor.matmul(
                        X_ps[:, i, :],
                        lhsT=hbT[:, c * 128 : (c + 1) * 128],
                        rhs=wcat_s[:],
                        start=True,
                        stop=True,
                    )
                nc.vector.tensor_copy(
                    gext_r[:, c0 : c0 + 2, :], X_ps[:, :, 0:GEXT]
                )
                nc.scalar.copy(
                    out=Xs_aj[:, c0 : c0 + 2, :], in_=X_ps[:, :, GEXT + H :]
                )
            # ones column per head block
            ones_view = bass.AP(
                tensor=gext_r.tensor,
                offset=gext_r.offset + D,
                ap=[gext_r.ap[0], [GEXT, C], [DP1, H]],
            )
            ones_src = bass.AP(
                tensor=onec.tensor, offset=onec.offset,
                ap=[onec.ap[0], [0, C], [0, H]],
            )
            nc.vector.tensor_copy(ones_view, ones_src)

            # E_j = exp(a_j), Fn_j = -exp(0.2 a_j)  (per-partition cols)
            Ej = ef_p.tile([128, C, H], F32, tag="ej")
            nc.scalar.activation(out=Ej[:], in_=Xs_aj[:], func=AF.Exp, scale=1.0 / KSC)
            Fnj = ef_p.tile([128, C, H], F32, tag="fnj")
            nc.scalar.activation(out=Fnj[:], in_=Xs_aj[:], func=AF.Exp, scale=0.2 / KSC)
            nc.vector.tensor_scalar(
                out=Fnj[:], in0=Fnj[:], scalar1=-1.0, scalar2=None, op0=ALU.mult
            )

            # stationary mega-tile SS[:, c, 0, h*17:] = E*[g|1], [:, c, 1, ...] = -F*[g|1]
            SS = ss_p.tile([128, C, H, 2 * DP1], BF16, tag="ss")
            SM = ss_p.tile([128, C, GEXT], BF16, tag="sm")
            for hh in range(H):
                gsl = gext_r[:, :, hh * DP1 : (hh + 1) * DP1]
                e_rep = _rep_ap(Ej, hh, Ej.ap[0], [[H, C], [0, DP1]])
                f_rep = _rep_ap(Fnj, hh, Fnj.ap[0], [[H, C], [0, DP1]])
                beng = nc.vector if hh % 2 == 0 else nc.gpsimd
                beng.tensor_tensor(
                    out=SS[:, :, hh, 0:DP1], in0=gsl, in1=e_rep, op=ALU.mult
                )
                beng.tensor_tensor(
                    out=SM[:, :, hh * DP1 : (hh + 1) * DP1],
                    in0=gsl, in1=f_rep, op=ALU.mult,
                )
            # mirror F blocks into SS's per-head [E|F] stationary (bf16 4x copy)
            sm_view = bass.AP(
                tensor=SM.tensor, offset=SM.offset,
                ap=[SM.ap[0], [GEXT, C], [DP1, H], [1, DP1]],
            )
            nc.vector.tensor_copy(
                bass.AP(
                    tensor=SS.tensor, offset=SS.offset + DP1,
                    ap=[SS.ap[0], [H * 2 * DP1, C], [2 * DP1, H], [1, DP1]],
                ),
                sm_view,
            )

            # a_i rows for every head -> bf16 -> DRAM (for broadcast DMA);
            # also transposed copy for per-partition E_i/F_i.
            XT_ps = xps_p.tile([H, N], F32, tag="prep")
            for half in range(2):
                sl = slice(half * 512, (half + 1) * 512)
                nc.tensor.matmul(
                    XT_ps[:, sl],
                    lhsT=wcat_s[:, GEXT : GEXT + H],
                    rhs=hbT[:, sl],
                    start=True,
                    stop=True,
                )
            ais_b = ef_p.tile([H, N], BF16, tag="aisb")
            nc.scalar.copy(out=ais_b[:], in_=XT_ps[:])
            nc.sync.dma_start(out=ai_scr[b], in_=ais_b[:])
            # E_i / -F_i per-partition (i-layout): PSUM ai -> exp via ACT
            aiT = ef_p.tile([H, N], F32, tag="aif")
            nc.scalar.copy(out=aiT[:], in_=XT_ps[:])
            # transpose [8, 128] chunks -> [128, C, 8]
            aiT_ps = xps_p.tile([128, C, H], F32, tag="prep")
            for c in range(C):
                nc.tensor.transpose(
                    aiT_ps[:, c, :], aiT[:, c * 128 : (c + 1) * 128],
                    id34_s[0:H, 0:H],
                )
            Gi = ef_p.tile([128, C, H], F32, tag="gyi")
            nc.scalar.activation(out=Gi[:], in_=aiT_ps[:], func=AF.Exp, scale=-0.8 / KSC)

            # M1: notm @ (-F*[g|1]) for all heads, 2 streams of 68 cols
            vm1 = m1_p.tile([128, C, 2, 4 * DP1], F32, tag="vm1")
            for s in range(2):
                ssl = slice(s * 4 * DP1, (s + 1) * 4 * DP1)
                M1_ps = xps_p.tile([4 * DP1, N], F32, tag="prep")
                for c in range(C):
                    for half in range(2):
                        sl = slice(half * 512, (half + 1) * 512)
                        nc.tensor.matmul(
                            M1_ps[:, sl],
                            lhsT=SM[:, c, ssl],
                            rhs=notm[:, c, sl],
                            start=(c == 0),
                            stop=(c == C - 1),
                        )
                m1sb = ut_p.tile([4 * DP1, N], F32, tag="m1sb")
                nc.scalar.copy(out=m1sb[:], in_=M1_ps[:])
                # one transpose per chunk into a single-bank PSUM tile: a
                # [128, C, 68] tile would cross the 2KB PSUM bank boundary
                # inside chunk 7's transpose output and corrupt cols 34:68
                for c in range(C):
                    VM1_ps = xps_p.tile([128, 4 * DP1], F32, tag="prep")
                    nc.tensor.transpose(
                        VM1_ps[:], m1sb[:, c * 128 : (c + 1) * 128], id68_s[:]
                    )
                    nc.vector.tensor_copy(vm1[:, c, s, :], VM1_ps[:])
            if DEBUG_DUMP:
                nc.sync.dma_start(out=dbg_vm1[b], in_=vm1[:])

            bcs = []
            for hh in range(H):
                bc = bc_p.tile([128, N], BF16)
                bcast_src = bass.AP(
                    tensor=ai_scr, offset=(b * H + hh) * N,
                    ap=[[0, 128], [1, N]],
                )
                nc.sync.dma_start(out=bc[:], in_=bcast_src)
                bcs.append(bc)

            return dict(notm=notm, gext=gext_r, aj=Xs_aj, SS=SS,
                        Gi=Gi, vm1=vm1, bcs=bcs)

        preps = {0: _prep(0)}

        for b in range(B_SH):
            g = preps.pop(b)
            notm, SS = g["notm"], g["SS"]
            out_b = ob_p.tile([128, C, H * D], F32)

            def postproc(h, UT_ps, k):
                # PSUM [34, N] -> SBUF, transpose to V [128, C, 34],
                # U = V_E + Gi*(VM1_h + V_F)  (V_F carries -F sign; Gi = F_i/E_i)
                ceng = {"A": nc.scalar, "D": nc.vector, "P": nc.gpsimd}[COPY_ENG[k]]
                UT_s = ut_p.tile([2 * DP1, N], F32, tag="uts")
                if COPY_ENG[k] == "A":
                    ceng.copy(out=UT_s[:], in_=UT_ps[:])
                else:
                    ceng.tensor_copy(UT_s[:], UT_ps[:])
                V_ps = vps_p.tile([128, C, 2 * DP1], F32, tag="vps")
                for c in range(C):
                    nc.tensor.transpose(
                        V_ps[:, c, :], UT_s[:, c * 128 : (c + 1) * 128],
                        id34_s[:],
                    )
                eng1 = nc.vector  # touches PSUM (V_ps) - GPSIMD cannot
                eng2 = nc.vector if PP_ENG[k % 4] == "D" else nc.gpsimd
                vm1_sl = g["vm1"][:, :, h // 4, (h % 4) * DP1 : (h % 4 + 1) * DP1]
                gi_rep = _rep_ap(g["Gi"], h, g["Gi"].ap[0], [[H, C], [0, DP1]])
                tmpF = pp_p.tile([128, C, DP1], F32, tag="tmpf")
                eng1.tensor_tensor(
                    out=tmpF[:], in0=V_ps[:, :, DP1:], in1=vm1_sl,
                    op=ALU.subtract,
                )
                tmpF2 = pp_p.tile([128, C, DP1], F32, tag="tmpf2")
                eng2.tensor_tensor(
                    out=tmpF2[:], in0=tmpF[:], in1=gi_rep, op=ALU.mult
                )
                U = pp_p.tile([128, C, DP1], F32, tag="u")
                eng1.tensor_tensor(
                    out=U[:], in0=V_ps[:, :, 0:DP1], in1=tmpF2[:], op=ALU.add
                )
                rc = pp_p.tile([128, C], F32, tag="rc")
                nc.vector.reciprocal(rc[:], U[:, :, D])
                rc_rep = bass.AP(
                    tensor=rc.tensor, offset=rc.offset,
                    ap=[rc.ap[0], rc.ap[-1], [0, D]],
                )
                us = pp_p.tile([128, C, D], F32, tag="us")
                eng2.tensor_tensor(
                    out=us[:], in0=U[:, :, 0:D], in1=rc_rep, op=ALU.mult
                )
                if k % 2 == 0:
                    nc.scalar.activation(
                        out=out_b[:, :, h * D : (h + 1) * D], in_=us[:],
                        func=AF.Relu,
                    )
                else:
                    nc.vector.tensor_scalar(
                        out=out_b[:, :, h * D : (h + 1) * D], in0=us[:],
                        scalar1=0.0, scalar2=None, op0=ALU.max,
                    )

            pending = []

            for h in range(H):
                k = b * H + h
                bc = g["bcs"][h]

                def aj_col(c):
                    return bass.AP(
                        tensor=g["aj"].tensor,
                        offset=g["aj"].offset + c * H + h,
                        ap=[g["aj"].ap[0], [1, 1]],
                    )

                UT_ps = ups_p.tile([2 * DP1, N], F32, tag="utps")
                for g0 in range(0, C, G):
                    flavor = FLAVORS[(k * (C // G) + g0 // G) % NSLOT]
                    Zt = sc_p.tile(
                        [128, G, N], BF16, tag="sc", name=f"z_{b}_{h}_{g0}"
                    )
                    if flavor[0] == "D":
                        # Z = (bc + a_j) >= 0 on DVE: bf16 -> 4x mode
                        for i in range(G):
                            nc.vector.tensor_scalar(
                                out=Zt[:, i, :], in0=bc[:],
                                scalar1=aj_col(g0 + i), scalar2=0.0,
                                op0=ALU.add, op1=ALU.is_ge,
                            )
                    else:
                        # sigmoid(K*(a_i + a_j)) -> {0,1} except a ~1e-4 band
                        # where the E and F branches agree anyway
                        for i in range(G):
                            nc.scalar.activation(
                                out=Zt[:, i, :], in_=bc[:], func=AF.Sigmoid,
                                bias=aj_col(g0 + i), scale=1.0,
                            )

                    meng = nc.vector if flavor[1] == "D" else nc.gpsimd
                    for i in range(G):
                        meng.tensor_tensor(
                            out=Zt[:, i, :], in0=Zt[:, i, :],
                            in1=notm[:, g0 + i, :], op=ALU.mult,
                        )

                    for i in range(G):
                        c = g0 + i
                        lhs = SS[:, c, h, :]
                        for half in range(2):
                            sl = slice(half * 512, (half + 1) * 512)
                            nc.tensor.matmul(
                                UT_ps[:, sl],
                                lhsT=lhs,
                                rhs=Zt[:, i, sl],
                                start=(c == 0),
                                stop=(c == C - 1),
                            )

                if pending:
                    postproc(*pending.pop())
                pending.append((h, UT_ps, k))
                if h == 0 and b + 1 < B_SH:
                    preps[b + 1] = _prep(b + 1)

            if pending:
                postproc(*pending.pop())
            nc.sync.dma_start(
                out=out[b].rearrange("(c p) d -> p c d", p=128), in_=out_b[:]
            )

    _wrap_to_json(nc)
    return nc


_NC_CACHE = None


def kernel(h, W, Wal, War, mask):
    global _NC_CACHE
    from concourse.bass_utils import run_bass_kernel_spmd

    h = np.asarray(h, dtype=np.float32)
    W = np.asarray(W, dtype=np.float32)
    Wal = np.asarray(Wal, dtype=np.float32)
    War = np.asarray(War, dtype=np.float32)
    import ml_dtypes

    notm_b16 = (~np.asarray(mask, dtype=bool)).astype(ml_dtypes.bfloat16)

    # Fold weights: wcat = [per-head (W_h | 0)] + [W@Wal] + [W@War]
    wcat = np.zeros((NI, WCOLS), dtype=np.float32)
    for hh in range(H):
        wcat[:, hh * DP1 : hh * DP1 + D] = W[hh]
        wcat[:, GEXT + hh] = KSC * (W[hh] @ Wal[hh, :, 0])
        wcat[:, GEXT + H + hh] = KSC * (W[hh] @ War[hh, :, 0])

    hT = np.ascontiguousarray(h.transpose(0, 2, 1))            # (B, I, N)
    notmT = np.ascontiguousarray(notm_b16.transpose(0, 2, 1))  # (B, j, i)
    id34 = np.eye(2 * DP1, dtype=np.float32)
    id68 = np.eye(4 * DP1, dtype=np.float32)

    if _NC_CACHE is None:
        _NC_CACHE = build_nc()
    nc = _NC_CACHE

    in_maps = []
    for core in range(NCORES):
        sl = slice(core * B_SH, (core + 1) * B_SH)
        in_maps.append(
            {
                "hT": np.ascontiguousarray(hT[sl]),
                "notmT": np.ascontiguousarray(notmT[sl]),
                "wcat": wcat,
                "id34": id34,
                "id68": id68,
            }
        )

    res = run_bass_kernel_spmd(nc, in_maps, list(range(NCORES)))
    if DEBUG_DUMP:
        global DBG_VM1
        DBG_VM1 = np.concatenate(
            [res.results[i]["dbg_vm1"] for i in range(NCORES)], axis=0
        )
    out = np.concatenate([res.results[i]["out"] for i in range(NCORES)], axis=0)
    return out.astype(np.float32)
